# revision 5
# baseline (speedup 1.0000x reference)
"""Trainium2 Bass kernel for nn_DeepModel_multi_12945031430869.

Computes, for heads h in 0..31:
    y[:, h] = relu(x @ W1[h] + b1[h]) @ W2[h] + b2[h]
    out[:, h*513:(h+1)*513] = [x, y[:, h]]          # [4096, 16416]

Sharding: head-parallel across 8 NeuronCores (4 heads per core). Each core
produces its own [4096, 4*513] column block; the host concatenates them.

Per-core device program (v3):
  - GEMM1 in bf16 on the PE array (fp32r measured ~2x slower). Per
    (head, row-tile) two [128, 1024] PSUM half-tiles (2 banks each,
    pool bufs=4 -> all 8 banks, 4 half-tiles in flight) are filled by
    8 matmuls each (k-outer: one stationary x-block streams 2 rhs tiles).
  - Epilogue folds |w2| into W1 columns (host), sorted by descending w2:
    cols [0,960) all-positive folded, [960,1088) mixed "M" region kept
    raw, [1088,2048) all-negative folded. Three DVE ops per (head,rt):
      bigA: (ps_a * +1) max sb1 -> sc[:, 0:1024],  accum -> accA
            (folded cols give C_f - |w2|b1; raw M cols give m=max(z,-b1))
      bigB: (ps_a * -1) min sb1 -> sc[:,1024:2048], accum -> accB
            (folded cols give C_f + |w2|b1; raw M cols give -m)
      M2:   sc[:, 960:1088] * v -> accum accM,  v = w2-1 (Ma) | -(w2+1) (Mb)
            so the m-terms total w2*m exactly.
    Constant residues sum to b2eff = b2 + sum_f w2_f b1_f; applied via
    Scalar activation(Identity, bias=b2eff/3, accum_out=y) -> ob col 512.
  - Next head's W1 DMA is issued during rt==0 of the current head so the
    2 MB transfer never queues behind the head's 64 output-block DMAs.
"""

import numpy as np

N = 4096
D_IN = 512
D_H = 2048
USED = 32
NCORES = 8
HPC = USED // NCORES  # heads per core = 4
KT = D_IN // 128      # k tiles = 4
RT = N // 128         # row tiles = 32
HB = D_H // 2         # psum half width = 1024

_PROGS = {}


def _build(mlo, mhi):
    import concourse.tile as tile
    import concourse.mybir as mybir
    from concourse import bacc

    f32 = mybir.dt.float32
    bf16 = mybir.dt.bfloat16
    mw = mhi - mlo

    nc = bacc.Bacc("TRN2", target_bir_lowering=False, debug=False)

    xT_d = nc.dram_tensor("xT", [KT, 128, N], bf16, kind="ExternalInput").ap()
    x_d = nc.dram_tensor("x", [N, D_IN], f32, kind="ExternalInput").ap()
    w1_d = nc.dram_tensor("w1", [HPC, 128, KT * D_H], bf16, kind="ExternalInput").ap()
    sb1_d = nc.dram_tensor("sb1", [128, HPC * D_H], f32, kind="ExternalInput").ap()
    w2m_d = nc.dram_tensor("w2m", [128, HPC * mw], f32, kind="ExternalInput").ap()
    b2_d = nc.dram_tensor("b2r", [128, HPC], f32, kind="ExternalInput").ap()
    out_d = nc.dram_tensor("out", [N, HPC * 513], f32, kind="ExternalOutput").ap()

    with tile.TileContext(nc) as tc:
        with tc.tile_pool(name="xt", bufs=1) as xtp, \
             tc.tile_pool(name="cst", bufs=1) as cst, \
             tc.tile_pool(name="w1p", bufs=2) as w1p, \
             tc.tile_pool(name="ps", bufs=4, space="PSUM") as pp, \
             tc.tile_pool(name="scr", bufs=3) as scr, \
             tc.tile_pool(name="sm", bufs=3) as smp, \
             tc.tile_pool(name="acc", bufs=6) as accp, \
             tc.tile_pool(name="j3", bufs=6) as j3p, \
             tc.tile_pool(name="ob", bufs=6) as obp:

            xts = []
            for k in range(KT):
                t = xtp.tile([128, N], bf16, tag=f"x{k}")
                nc.sync.dma_start(t[:], xT_d[k])
                xts.append(t)
            sb1 = cst.tile([128, HPC * D_H], f32, tag="sb1")
            nc.sync.dma_start(sb1[:], sb1_d[:])
            w2m = cst.tile([128, HPC * mw], f32, tag="w2m")
            nc.sync.dma_start(w2m[:], w2m_d[:])
            b2r = cst.tile([128, HPC], f32, tag="b2r")
            nc.sync.dma_start(b2r[:], b2_d[:])

            mx = mybir.AluOpType.max
            mn = mybir.AluOpType.min
            mult = mybir.AluOpType.mult
            ident = mybir.ActivationFunctionType.Identity

            w1ts = [w1p.tile([128, KT * D_H], bf16, tag="w1", name="w1t0")]
            nc.sync.dma_start(w1ts[0][:], w1_d[0])

            for h in range(HPC):
                w1t = w1ts[h]
                for rt in range(RT):
                    rs = rt * 128
                    ob = obp.tile([128, 513], f32, tag="ob")
                    nc.sync.dma_start(ob[:, 0:512], x_d[rs:rs + 128, :])
                    halves = []
                    for half in range(2):
                        ps = pp.tile([128, HB], f32, tag="ps")
                        for k in range(KT):
                            for tt in range(2):
                                c = half * HB + tt * 512
                                nc.tensor.matmul(
                                    ps[:, tt * 512:(tt + 1) * 512],
                                    lhsT=xts[k][:, rs:rs + 128],
                                    rhs=w1t[:, k * D_H + c:k * D_H + c + 512],
                                    start=(k == 0),
                                    stop=(k == KT - 1),
                                )
                        halves.append(ps)
                    if rt == 0 and h + 1 < HPC:
                        w1ts.append(w1p.tile([128, KT * D_H], bf16, tag="w1", name=f"w1t{h+1}"))
                        nc.sync.dma_start(w1ts[h + 1][:], w1_d[h + 1])
                    c0 = h * D_H
                    acc = accp.tile([128, 3], f32, tag="acc")
                    sc = scr.tile([128, D_H], f32, tag="sc")
                    nc.vector.scalar_tensor_tensor(
                        out=sc[:, 0:HB],
                        in0=halves[0][:],
                        scalar=1.0,
                        in1=sb1[:, c0:c0 + HB],
                        op0=mult,
                        op1=mx,
                        accum_out=acc[:, 0:1],
                    )
                    nc.vector.scalar_tensor_tensor(
                        out=sc[:, HB:D_H],
                        in0=halves[1][:],
                        scalar=-1.0,
                        in1=sb1[:, c0 + HB:c0 + D_H],
                        op0=mult,
                        op1=mn,
                        accum_out=acc[:, 1:2],
                    )
                    sm = smp.tile([128, mw], f32, tag="sm")
                    nc.vector.scalar_tensor_tensor(
                        out=sm[:],
                        in0=sc[:, mlo:mhi],
                        scalar=1.0,
                        in1=w2m[:, h * mw:(h + 1) * mw],
                        op0=mult,
                        op1=mult,
                        accum_out=acc[:, 2:3],
                    )
                    j3 = j3p.tile([128, 3], f32, tag="j3")
                    nc.scalar.activation(
                        j3[:], acc[:], ident,
                        bias=b2r[:, h:h + 1], scale=1.0,
                        accum_out=ob[:, 512:513],
                    )
                    nc.sync.dma_start(
                        out_d[rs:rs + 128, h * 513:(h + 1) * 513], ob[:]
                    )

    nc.compile()
    return nc


def _get_program(mlo, mhi):
    key = (mlo, mhi)
    if key not in _PROGS:
        _PROGS[key] = _build(mlo, mhi)
    return _PROGS[key]


def kernel(x, W1, b1, W2, b2):
    import ml_dtypes
    from concourse.bass_utils import run_bass_kernel_spmd

    x = np.asarray(x, dtype=np.float32)
    W1 = np.asarray(W1, dtype=np.float32)
    b1 = np.asarray(b1, dtype=np.float32)
    W2 = np.asarray(W2, dtype=np.float32)
    b2 = np.asarray(b2, dtype=np.float32)

    # M region must contain every head's sign boundary; widen if needed
    P = (W2[:USED] > 0).sum(axis=1)
    mlo, mhi = 960, 1088
    if P.min() < mlo:
        mlo = max(0, int(P.min()) // 64 * 64)
    if P.max() > mhi:
        mhi = min(D_H, -(-int(P.max()) // 64) * 64)
    assert mlo < 1024 < mhi
    mw = mhi - mlo

    nc = _get_program(mlo, mhi)

    xT = np.ascontiguousarray(x.T).astype(ml_dtypes.bfloat16).reshape(KT, 128, N)

    in_maps = []
    for c in range(NCORES):
        w1heads = []
        sb1cols = np.empty(HPC * D_H, dtype=np.float32)
        w2mcols = np.empty(HPC * mw, dtype=np.float32)
        b2eff = np.empty(HPC, dtype=np.float32)
        for i in range(HPC):
            h = HPC * c + i
            w2 = W2[h]
            order = np.argsort(-w2, kind="stable")  # descending w2
            w2s = w2[order]
            b1s = b1[h][order]
            assert w2s[mlo - 1] > 0 and w2s[mhi] < 0, "region overflow"
            aw = np.abs(w2s)
            scale = aw.copy()
            scale[mlo:mhi] = 1.0  # M region stays unfolded
            w1s = W1[h][:, order] * scale[None, :]
            sb = np.empty(D_H, dtype=np.float32)
            sb[:mlo] = -aw[:mlo] * b1s[:mlo]
            sb[mlo:1024] = -b1s[mlo:1024]          # Ma (raw, in bigA)
            sb[1024:mhi] = b1s[1024:mhi]           # Mb (raw, in bigB)
            sb[mhi:] = aw[mhi:] * b1s[mhi:]
            sb1cols[i * D_H:(i + 1) * D_H] = sb
            w2mcols[i * mw:i * mw + (1024 - mlo)] = w2s[mlo:1024] - 1.0
            w2mcols[i * mw + (1024 - mlo):(i + 1) * mw] = -(w2s[1024:mhi] + 1.0)
            b2eff[i] = (
                b2[h].astype(np.float64)
                + np.dot(w2.astype(np.float64), b1[h].astype(np.float64))
            ) / 3.0
            # [512, 2048] -> [128 part, KT*D_H] with cols = k*D_H + j
            w1heads.append(
                np.ascontiguousarray(
                    w1s.reshape(KT, 128, D_H).transpose(1, 0, 2)
                ).reshape(128, KT * D_H)
            )
        in_maps.append({
            "xT": xT,
            "x": x,
            "w1": np.ascontiguousarray(
                np.stack(w1heads, axis=0).astype(ml_dtypes.bfloat16)
            ),
            "sb1": np.ascontiguousarray(
                np.broadcast_to(sb1cols.reshape(1, -1), (128, HPC * D_H))
            ),
            "w2m": np.ascontiguousarray(
                np.broadcast_to(w2mcols.reshape(1, -1), (128, HPC * mw))
            ),
            "b2r": np.ascontiguousarray(
                np.broadcast_to(b2eff.reshape(1, -1), (128, HPC))
            ),
        })

    import os
    trace = os.environ.get("BASS_KERNEL_TRACE") == "1"
    if trace:
        import sys
        sys.path.insert(0, "/tmp")
        try:
            import axon_shim
            axon_shim.install()
        except Exception:
            trace = False
    res = run_bass_kernel_spmd(nc, in_maps, list(range(NCORES)), trace=trace)
    kernel.last_result = res

    return np.concatenate([res.results[c]["out"] for c in range(NCORES)], axis=1)


# revision 7
# speedup vs baseline: 1.1580x; 1.1580x over previous
"""Trainium2 Bass kernel for nn_DeepModel_multi_12945031430869.

Computes, for heads h in 0..31:
    y[:, h] = relu(x @ W1[h] + b1[h]) @ W2[h] + b2[h]
    out[:, h*513:(h+1)*513] = [x, y[:, h]]          # [4096, 16416]

Sharding: head-parallel across 8 NeuronCores (4 heads per core). Each core
produces its own [4096, 4*513] column block; the host concatenates them and
inserts the y columns.

Per-core device program (v4):
  - GEMM1 in bf16 on the PE array. Per (head, row-tile) two [128, 1024]
    PSUM half-tiles (2 banks each, pool bufs=4 -> all 8 banks). Matmul
    order is k-outer across BOTH halves so each stationary x-block feeds
    4 matmuls (fewer exposed weight reloads).
  - Epilogue folds |w2| into W1 columns (host), sorted by descending w2:
    cols [0,960) all-positive folded, [960,1088) mixed "M" region raw,
    [1088,2048) all-negative folded. Three DVE ops per (head, rt):
      bigA: (ps_a * +1) max sb1 -> sc[:, 0:1024],   accum -> accA
      bigB: (ps_b * -1) min sb1 -> sc[:, 1024:2048], accum -> accB
      M2:   sc[:, 960:1088] * v -> accum accM,  v = w2-1 (Ma) | -(w2+1) (Mb)
    Constant residues sum to b2eff = b2 + sum_f w2_f b1_f; applied by the
    Scalar engine: activation(Identity, bias=b2eff/3, accum_out) writes y
    into a per-head [128, 32] staging tile (col = rt) -> one DMA per head
    to the compact y_d DRAM tensor. Host scatters y into the out columns.
  - The output x-blocks never touch the compute pipeline: 128 independent
    DRAM->DRAM DMAs (issued spread across head 0's loop) copy x into the
    out blocks. This removes the ob-tile DMA-in -> activation -> DMA-out
    serialization that throttled v2/v3 (Sync-queue waits of 10-40 us).
  - Next head's W1 DMA is issued during rt==0 of the current head.
"""

import numpy as np

N = 4096
D_IN = 512
D_H = 2048
USED = 32
NCORES = 8
HPC = USED // NCORES  # heads per core = 4
KT = D_IN // 128      # k tiles = 4
RT = N // 128         # row tiles = 32
HB = D_H // 2         # psum half width = 1024

_PROGS = {}


def _build(mlo, mhi):
    import concourse.tile as tile
    import concourse.mybir as mybir
    from concourse import bacc

    f32 = mybir.dt.float32
    bf16 = mybir.dt.bfloat16
    mw = mhi - mlo

    nc = bacc.Bacc("TRN2", target_bir_lowering=False, debug=False)

    xT_d = nc.dram_tensor("xT", [KT, 128, N], bf16, kind="ExternalInput").ap()
    x_d = nc.dram_tensor("x", [N, D_IN], f32, kind="ExternalInput").ap()
    w1_d = nc.dram_tensor("w1", [HPC, 128, KT * D_H], bf16, kind="ExternalInput").ap()
    sb1_d = nc.dram_tensor("sb1", [128, HPC * D_H], f32, kind="ExternalInput").ap()
    w2m_d = nc.dram_tensor("w2m", [128, HPC * mw], f32, kind="ExternalInput").ap()
    b2_d = nc.dram_tensor("b2r", [128, HPC], f32, kind="ExternalInput").ap()
    out_d = nc.dram_tensor("out", [N, HPC * 513], f32, kind="ExternalOutput").ap()
    y_d = nc.dram_tensor("yst", [HPC, 128, RT], f32, kind="ExternalOutput").ap()

    with tile.TileContext(nc) as tc:
        with tc.tile_pool(name="xt", bufs=1) as xtp, \
             tc.tile_pool(name="cst", bufs=1) as cst, \
             tc.tile_pool(name="w1p", bufs=2) as w1p, \
             tc.tile_pool(name="ys", bufs=2) as ysp, \
             tc.tile_pool(name="ps", bufs=4, space="PSUM") as pp, \
             tc.tile_pool(name="scr", bufs=3) as scr, \
             tc.tile_pool(name="sm", bufs=3) as smp, \
             tc.tile_pool(name="acc", bufs=6) as accp, \
             tc.tile_pool(name="j3", bufs=6) as j3p:

            xts = []
            for k in range(KT):
                t = xtp.tile([128, N], bf16, tag=f"x{k}")
                nc.sync.dma_start(t[:], xT_d[k])
                xts.append(t)
            sb1 = cst.tile([128, HPC * D_H], f32, tag="sb1")
            nc.sync.dma_start(sb1[:], sb1_d[:])
            w2m = cst.tile([128, HPC * mw], f32, tag="w2m")
            nc.sync.dma_start(w2m[:], w2m_d[:])
            b2r = cst.tile([128, HPC], f32, tag="b2r")
            nc.sync.dma_start(b2r[:], b2_d[:])

            mx = mybir.AluOpType.max
            mn = mybir.AluOpType.min
            mult = mybir.AluOpType.mult
            ident = mybir.ActivationFunctionType.Identity

            w1ts = [w1p.tile([128, KT * D_H], bf16, tag="w1", name="w1t0")]
            nc.sync.dma_start(w1ts[0][:], w1_d[0])

            for h in range(HPC):
                w1t = w1ts[h]
                ys = ysp.tile([128, RT], f32, tag="ys", name=f"ys{h}")
                for rt in range(RT):
                    rs = rt * 128
                    if h == 0:
                        # independent DRAM->DRAM x-block copies, spread out
                        for hh in range(HPC):
                            nc.sync.dma_start(
                                out_d[rs:rs + 128, hh * 513:hh * 513 + 512],
                                x_d[rs:rs + 128, :],
                            )
                    ps_a = pp.tile([128, HB], f32, tag="ps", name=f"psa{h}_{rt}")
                    ps_b = pp.tile([128, HB], f32, tag="ps", name=f"psb{h}_{rt}")
                    for k in range(KT):
                        for tt in range(4):
                            ps = ps_a if tt < 2 else ps_b
                            nc.tensor.matmul(
                                ps[:, (tt % 2) * 512:(tt % 2) * 512 + 512],
                                lhsT=xts[k][:, rs:rs + 128],
                                rhs=w1t[:, k * D_H + tt * 512:k * D_H + (tt + 1) * 512],
                                start=(k == 0),
                                stop=(k == KT - 1),
                            )
                    if rt == 0 and h + 1 < HPC:
                        w1ts.append(w1p.tile([128, KT * D_H], bf16, tag="w1",
                                             name=f"w1t{h + 1}"))
                        nc.sync.dma_start(w1ts[h + 1][:], w1_d[h + 1])
                    c0 = h * D_H
                    acc = accp.tile([128, 3], f32, tag="acc")
                    sc = scr.tile([128, D_H], f32, tag="sc")
                    nc.vector.scalar_tensor_tensor(
                        out=sc[:, 0:HB],
                        in0=ps_a[:],
                        scalar=1.0,
                        in1=sb1[:, c0:c0 + HB],
                        op0=mult,
                        op1=mx,
                        accum_out=acc[:, 0:1],
                    )
                    nc.vector.scalar_tensor_tensor(
                        out=sc[:, HB:D_H],
                        in0=ps_b[:],
                        scalar=-1.0,
                        in1=sb1[:, c0 + HB:c0 + D_H],
                        op0=mult,
                        op1=mn,
                        accum_out=acc[:, 1:2],
                    )
                    sm = smp.tile([128, mw], f32, tag="sm")
                    nc.vector.scalar_tensor_tensor(
                        out=sm[:],
                        in0=sc[:, mlo:mhi],
                        scalar=1.0,
                        in1=w2m[:, h * mw:(h + 1) * mw],
                        op0=mult,
                        op1=mult,
                        accum_out=acc[:, 2:3],
                    )
                    j3 = j3p.tile([128, 3], f32, tag="j3")
                    nc.scalar.activation(
                        j3[:], acc[:], ident,
                        bias=b2r[:, h:h + 1], scale=1.0,
                        accum_out=ys[:, rt:rt + 1],
                    )
                nc.sync.dma_start(y_d[h], ys[:])

    nc.compile()
    return nc


def _get_program(mlo, mhi):
    key = (mlo, mhi)
    if key not in _PROGS:
        _PROGS[key] = _build(mlo, mhi)
    return _PROGS[key]


def kernel(x, W1, b1, W2, b2):
    import ml_dtypes
    from concourse.bass_utils import run_bass_kernel_spmd

    x = np.asarray(x, dtype=np.float32)
    W1 = np.asarray(W1, dtype=np.float32)
    b1 = np.asarray(b1, dtype=np.float32)
    W2 = np.asarray(W2, dtype=np.float32)
    b2 = np.asarray(b2, dtype=np.float32)

    # M region must contain every head's sign boundary; widen if needed
    P = (W2[:USED] > 0).sum(axis=1)
    mlo, mhi = 960, 1088
    if P.min() < mlo:
        mlo = max(0, int(P.min()) // 64 * 64)
    if P.max() > mhi:
        mhi = min(D_H, -(-int(P.max()) // 64) * 64)
    assert mlo < 1024 < mhi
    mw = mhi - mlo

    nc = _get_program(mlo, mhi)

    xT = np.ascontiguousarray(x.T).astype(ml_dtypes.bfloat16).reshape(KT, 128, N)

    in_maps = []
    for c in range(NCORES):
        w1heads = []
        sb1cols = np.empty(HPC * D_H, dtype=np.float32)
        w2mcols = np.empty(HPC * mw, dtype=np.float32)
        b2eff = np.empty(HPC, dtype=np.float32)
        for i in range(HPC):
            h = HPC * c + i
            w2 = W2[h]
            order = np.argsort(-w2, kind="stable")  # descending w2
            w2s = w2[order]
            b1s = b1[h][order]
            assert w2s[mlo - 1] > 0 and w2s[mhi] < 0, "region overflow"
            aw = np.abs(w2s)
            scale = aw.copy()
            scale[mlo:mhi] = 1.0  # M region stays unfolded
            w1s = W1[h][:, order] * scale[None, :]
            sb = np.empty(D_H, dtype=np.float32)
            sb[:mlo] = -aw[:mlo] * b1s[:mlo]
            sb[mlo:1024] = -b1s[mlo:1024]          # Ma (raw, in bigA)
            sb[1024:mhi] = b1s[1024:mhi]           # Mb (raw, in bigB)
            sb[mhi:] = aw[mhi:] * b1s[mhi:]
            sb1cols[i * D_H:(i + 1) * D_H] = sb
            w2mcols[i * mw:i * mw + (1024 - mlo)] = w2s[mlo:1024] - 1.0
            w2mcols[i * mw + (1024 - mlo):(i + 1) * mw] = -(w2s[1024:mhi] + 1.0)
            b2eff[i] = (
                b2[h].astype(np.float64)
                + np.dot(w2.astype(np.float64), b1[h].astype(np.float64))
            ) / 3.0
            # [512, 2048] -> [128 part, KT*D_H] with cols = k*D_H + j
            w1heads.append(
                np.ascontiguousarray(
                    w1s.reshape(KT, 128, D_H).transpose(1, 0, 2)
                ).reshape(128, KT * D_H)
            )
        in_maps.append({
            "xT": xT,
            "x": x,
            "w1": np.ascontiguousarray(
                np.stack(w1heads, axis=0).astype(ml_dtypes.bfloat16)
            ),
            "sb1": np.ascontiguousarray(
                np.broadcast_to(sb1cols.reshape(1, -1), (128, HPC * D_H))
            ),
            "w2m": np.ascontiguousarray(
                np.broadcast_to(w2mcols.reshape(1, -1), (128, HPC * mw))
            ),
            "b2r": np.ascontiguousarray(
                np.broadcast_to(b2eff.reshape(1, -1), (128, HPC))
            ),
        })

    import os
    trace = os.environ.get("BASS_KERNEL_TRACE") == "1"
    if trace:
        import sys
        sys.path.insert(0, "/tmp")
        try:
            import axon_shim
            axon_shim.install()
        except Exception:
            trace = False
    res = run_bass_kernel_spmd(nc, in_maps, list(range(NCORES)), trace=trace)
    kernel.last_result = res

    out = np.concatenate([res.results[c]["out"] for c in range(NCORES)], axis=1)
    for c in range(NCORES):
        yst = res.results[c]["yst"]  # [HPC, 128, RT]
        for i in range(HPC):
            # y[n] for n = rt*128 + p  lives at yst[i, p, rt]
            out[:, (c * HPC + i) * 513 + 512] = yst[i].T.reshape(N)
    return out


# revision 9
# speedup vs baseline: 1.5967x; 1.3789x over previous
"""Trainium2 Bass kernel for nn_DeepModel_multi_12945031430869.

Computes, for heads h in 0..31:
    y[:, h] = relu(x @ W1[h] + b1[h]) @ W2[h] + b2[h]
    out[:, h*513:(h+1)*513] = [x, y[:, h]]          # [4096, 16416]

Sharding: head-parallel across 8 NeuronCores (4 heads per core). Each core
produces its own [4096, 4*513] column block; the host concatenates them and
inserts the y columns.

Per-core device program (v4):
  - GEMM1 in bf16 on the PE array. Per (head, row-tile) two [128, 1024]
    PSUM half-tiles (2 banks each, pool bufs=4 -> all 8 banks). Matmul
    order is k-outer across BOTH halves so each stationary x-block feeds
    4 matmuls (fewer exposed weight reloads).
  - Epilogue folds |w2| into W1 columns (host), sorted by descending w2:
    cols [0,960) all-positive folded, [960,1088) mixed "M" region raw,
    [1088,2048) all-negative folded. Three DVE ops per (head, rt):
      bigA: (ps_a * +1) max sb1 -> sc[:, 0:1024],   accum -> accA
      bigB: (ps_b * -1) min sb1 -> sc[:, 1024:2048], accum -> accB
      M2:   sc[:, 960:1088] * v -> accum accM,  v = w2-1 (Ma) | -(w2+1) (Mb)
    Constant residues sum to b2eff = b2 + sum_f w2_f b1_f; applied by the
    Scalar engine: activation(Identity, bias=b2eff/3, accum_out) writes y
    into a per-head [128, 32] staging tile (col = rt) -> one DMA per head
    to the compact y_d DRAM tensor. Host scatters y into the out columns.
  - The output x-blocks never touch the compute pipeline: 128 independent
    DRAM->DRAM DMAs (issued spread across head 0's loop) copy x into the
    out blocks. This removes the ob-tile DMA-in -> activation -> DMA-out
    serialization that throttled v2/v3 (Sync-queue waits of 10-40 us).
  - Next head's W1 DMA is issued during rt==0 of the current head.
"""

import numpy as np

N = 4096
D_IN = 512
D_H = 2048
USED = 32
NCORES = 8
HPC = USED // NCORES  # heads per core = 4
KT = D_IN // 128      # k tiles = 4
RT = N // 128         # row tiles = 32
HB = D_H // 2         # psum half width = 1024

_PROGS = {}


def _build(mlo, mhi):
    import concourse.tile as tile
    import concourse.mybir as mybir
    from concourse import bacc

    f32 = mybir.dt.float32
    bf16 = mybir.dt.bfloat16
    mw = mhi - mlo

    nc = bacc.Bacc("TRN2", target_bir_lowering=False, debug=False)

    xT_d = nc.dram_tensor("xT", [KT, 128, N], bf16, kind="ExternalInput").ap()
    x_d = nc.dram_tensor("x", [N, D_IN], f32, kind="ExternalInput").ap()
    w1_d = nc.dram_tensor("w1", [HPC, 128, KT * D_H], bf16, kind="ExternalInput").ap()
    sb1_d = nc.dram_tensor("sb1", [128, HPC * D_H], f32, kind="ExternalInput").ap()
    w2m_d = nc.dram_tensor("w2m", [128, HPC * mw], f32, kind="ExternalInput").ap()
    b2_d = nc.dram_tensor("b2r", [128, HPC], f32, kind="ExternalInput").ap()
    out_d = nc.dram_tensor("out", [N, HPC * 513], f32, kind="ExternalOutput").ap()
    y_d = nc.dram_tensor("yst", [HPC, 128, RT], f32, kind="ExternalOutput").ap()

    with tile.TileContext(nc) as tc:
        with tc.tile_pool(name="xt", bufs=1) as xtp, \
             tc.tile_pool(name="cst", bufs=1) as cst, \
             tc.tile_pool(name="w1p", bufs=2) as w1p, \
             tc.tile_pool(name="ys", bufs=2) as ysp, \
             tc.tile_pool(name="xc", bufs=4) as xcp, \
             tc.tile_pool(name="ps", bufs=4, space="PSUM") as pp, \
             tc.tile_pool(name="scr", bufs=3) as scr, \
             tc.tile_pool(name="sm", bufs=3) as smp, \
             tc.tile_pool(name="acc", bufs=6) as accp, \
             tc.tile_pool(name="j3", bufs=6) as j3p:

            xts = []
            for k in range(KT):
                t = xtp.tile([128, N], bf16, tag=f"x{k}")
                nc.sync.dma_start(t[:], xT_d[k])
                xts.append(t)
            sb1 = cst.tile([128, HPC * D_H], f32, tag="sb1")
            nc.sync.dma_start(sb1[:], sb1_d[:])
            w2m = cst.tile([128, HPC * mw], f32, tag="w2m")
            nc.sync.dma_start(w2m[:], w2m_d[:])
            b2r = cst.tile([128, HPC], f32, tag="b2r")
            nc.sync.dma_start(b2r[:], b2_d[:])

            mx = mybir.AluOpType.max
            mn = mybir.AluOpType.min
            mult = mybir.AluOpType.mult
            ident = mybir.ActivationFunctionType.Identity

            w1ts = [w1p.tile([128, KT * D_H], bf16, tag="w1", name="w1t0")]
            nc.sync.dma_start(w1ts[0][:], w1_d[0])

            for h in range(HPC):
                w1t = w1ts[h]
                ys = ysp.tile([128, RT], f32, tag="ys", name=f"ys{h}")
                for rt in range(RT):
                    rs = rt * 128
                    if h == 0:
                        # x-block copies staged through SBUF on the Scalar
                        # engine's DMA queue: no compute deps, and DRAM->DRAM
                        # DMA (25 GB/s) is 14x slower than two staged hops
                        xst = xcp.tile([128, 512], f32, tag="xst")
                        nc.scalar.dma_start(xst[:], x_d[rs:rs + 128, :])
                        for hh in range(HPC):
                            nc.scalar.dma_start(
                                out_d[rs:rs + 128, hh * 513:hh * 513 + 512],
                                xst[:],
                            )
                    ps_a = pp.tile([128, HB], f32, tag="ps", name=f"psa{h}_{rt}")
                    ps_b = pp.tile([128, HB], f32, tag="ps", name=f"psb{h}_{rt}")
                    for k in range(KT):
                        for tt in range(4):
                            ps = ps_a if tt < 2 else ps_b
                            nc.tensor.matmul(
                                ps[:, (tt % 2) * 512:(tt % 2) * 512 + 512],
                                lhsT=xts[k][:, rs:rs + 128],
                                rhs=w1t[:, k * D_H + tt * 512:k * D_H + (tt + 1) * 512],
                                start=(k == 0),
                                stop=(k == KT - 1),
                            )
                    if rt == 0 and h + 1 < HPC:
                        w1ts.append(w1p.tile([128, KT * D_H], bf16, tag="w1",
                                             name=f"w1t{h + 1}"))
                        nc.sync.dma_start(w1ts[h + 1][:], w1_d[h + 1])
                    c0 = h * D_H
                    acc = accp.tile([128, 3], f32, tag="acc")
                    sc = scr.tile([128, D_H], f32, tag="sc")
                    nc.vector.scalar_tensor_tensor(
                        out=sc[:, 0:HB],
                        in0=ps_a[:],
                        scalar=1.0,
                        in1=sb1[:, c0:c0 + HB],
                        op0=mult,
                        op1=mx,
                        accum_out=acc[:, 0:1],
                    )
                    nc.vector.scalar_tensor_tensor(
                        out=sc[:, HB:D_H],
                        in0=ps_b[:],
                        scalar=-1.0,
                        in1=sb1[:, c0 + HB:c0 + D_H],
                        op0=mult,
                        op1=mn,
                        accum_out=acc[:, 1:2],
                    )
                    sm = smp.tile([128, mw], f32, tag="sm")
                    nc.vector.scalar_tensor_tensor(
                        out=sm[:],
                        in0=sc[:, mlo:mhi],
                        scalar=1.0,
                        in1=w2m[:, h * mw:(h + 1) * mw],
                        op0=mult,
                        op1=mult,
                        accum_out=acc[:, 2:3],
                    )
                    j3 = j3p.tile([128, 3], f32, tag="j3")
                    nc.scalar.activation(
                        j3[:], acc[:], ident,
                        bias=b2r[:, h:h + 1], scale=1.0,
                        accum_out=ys[:, rt:rt + 1],
                    )
                nc.sync.dma_start(y_d[h], ys[:])

    nc.compile()
    return nc


def _get_program(mlo, mhi):
    key = (mlo, mhi)
    if key not in _PROGS:
        _PROGS[key] = _build(mlo, mhi)
    return _PROGS[key]


def kernel(x, W1, b1, W2, b2):
    import ml_dtypes
    from concourse.bass_utils import run_bass_kernel_spmd

    x = np.asarray(x, dtype=np.float32)
    W1 = np.asarray(W1, dtype=np.float32)
    b1 = np.asarray(b1, dtype=np.float32)
    W2 = np.asarray(W2, dtype=np.float32)
    b2 = np.asarray(b2, dtype=np.float32)

    # M region must contain every head's sign boundary; widen if needed
    P = (W2[:USED] > 0).sum(axis=1)
    mlo, mhi = 960, 1088
    if P.min() < mlo:
        mlo = max(0, int(P.min()) // 64 * 64)
    if P.max() > mhi:
        mhi = min(D_H, -(-int(P.max()) // 64) * 64)
    assert mlo < 1024 < mhi
    mw = mhi - mlo

    nc = _get_program(mlo, mhi)

    xT = np.ascontiguousarray(x.T).astype(ml_dtypes.bfloat16).reshape(KT, 128, N)

    in_maps = []
    for c in range(NCORES):
        w1heads = []
        sb1cols = np.empty(HPC * D_H, dtype=np.float32)
        w2mcols = np.empty(HPC * mw, dtype=np.float32)
        b2eff = np.empty(HPC, dtype=np.float32)
        for i in range(HPC):
            h = HPC * c + i
            w2 = W2[h]
            order = np.argsort(-w2, kind="stable")  # descending w2
            w2s = w2[order]
            b1s = b1[h][order]
            assert w2s[mlo - 1] > 0 and w2s[mhi] < 0, "region overflow"
            aw = np.abs(w2s)
            scale = aw.copy()
            scale[mlo:mhi] = 1.0  # M region stays unfolded
            w1s = W1[h][:, order] * scale[None, :]
            sb = np.empty(D_H, dtype=np.float32)
            sb[:mlo] = -aw[:mlo] * b1s[:mlo]
            sb[mlo:1024] = -b1s[mlo:1024]          # Ma (raw, in bigA)
            sb[1024:mhi] = b1s[1024:mhi]           # Mb (raw, in bigB)
            sb[mhi:] = aw[mhi:] * b1s[mhi:]
            sb1cols[i * D_H:(i + 1) * D_H] = sb
            w2mcols[i * mw:i * mw + (1024 - mlo)] = w2s[mlo:1024] - 1.0
            w2mcols[i * mw + (1024 - mlo):(i + 1) * mw] = -(w2s[1024:mhi] + 1.0)
            b2eff[i] = (
                b2[h].astype(np.float64)
                + np.dot(w2.astype(np.float64), b1[h].astype(np.float64))
            ) / 3.0
            # [512, 2048] -> [128 part, KT*D_H] with cols = k*D_H + j
            w1heads.append(
                np.ascontiguousarray(
                    w1s.reshape(KT, 128, D_H).transpose(1, 0, 2)
                ).reshape(128, KT * D_H)
            )
        in_maps.append({
            "xT": xT,
            "x": x,
            "w1": np.ascontiguousarray(
                np.stack(w1heads, axis=0).astype(ml_dtypes.bfloat16)
            ),
            "sb1": np.ascontiguousarray(
                np.broadcast_to(sb1cols.reshape(1, -1), (128, HPC * D_H))
            ),
            "w2m": np.ascontiguousarray(
                np.broadcast_to(w2mcols.reshape(1, -1), (128, HPC * mw))
            ),
            "b2r": np.ascontiguousarray(
                np.broadcast_to(b2eff.reshape(1, -1), (128, HPC))
            ),
        })

    import os
    trace = os.environ.get("BASS_KERNEL_TRACE") == "1"
    if trace:
        import sys
        sys.path.insert(0, "/tmp")
        try:
            import axon_shim
            axon_shim.install()
        except Exception:
            trace = False
    res = run_bass_kernel_spmd(nc, in_maps, list(range(NCORES)), trace=trace)
    kernel.last_result = res

    out = np.concatenate([res.results[c]["out"] for c in range(NCORES)], axis=1)
    for c in range(NCORES):
        yst = res.results[c]["yst"]  # [HPC, 128, RT]
        for i in range(HPC):
            # y[n] for n = rt*128 + p  lives at yst[i, p, rt]
            out[:, (c * HPC + i) * 513 + 512] = yst[i].T.reshape(N)
    return out


# revision 10
# speedup vs baseline: 1.9635x; 1.2297x over previous
"""Trainium2 Bass kernel for nn_DeepModel_multi_12945031430869.

Computes, for heads h in 0..31:
    y[:, h] = relu(x @ W1[h] + b1[h]) @ W2[h] + b2[h]
    out[:, h*513:(h+1)*513] = [x, y[:, h]]          # [4096, 16416]

Sharding: head-parallel across 8 NeuronCores (4 heads per core). Each core
produces its own [4096, 4*513] column block; the host concatenates them and
inserts the y columns.

Per-core device program (v4):
  - GEMM1 in bf16 on the PE array. Per (head, row-tile) two [128, 1024]
    PSUM half-tiles (2 banks each, pool bufs=4 -> all 8 banks). Matmul
    order is k-outer across BOTH halves so each stationary x-block feeds
    4 matmuls (fewer exposed weight reloads).
  - Epilogue folds |w2| into W1 columns (host), sorted by descending w2:
    cols [0,960) all-positive folded, [960,1088) mixed "M" region raw,
    [1088,2048) all-negative folded. Three DVE ops per (head, rt):
      bigA: (ps_a * +1) max sb1 -> sc[:, 0:1024],   accum -> accA
      bigB: (ps_b * -1) min sb1 -> sc[:, 1024:2048], accum -> accB
      M2:   sc[:, 960:1088] * v -> accum accM,  v = w2-1 (Ma) | -(w2+1) (Mb)
    Constant residues sum to b2eff = b2 + sum_f w2_f b1_f; applied by the
    Scalar engine: activation(Identity, bias=b2eff/3, accum_out) writes y
    into a per-head [128, 32] staging tile (col = rt) -> one DMA per head
    to the compact y_d DRAM tensor. Host scatters y into the out columns.
  - The output x-blocks never touch the compute pipeline: 128 independent
    DRAM->DRAM DMAs (issued spread across head 0's loop) copy x into the
    out blocks. This removes the ob-tile DMA-in -> activation -> DMA-out
    serialization that throttled v2/v3 (Sync-queue waits of 10-40 us).
  - Next head's W1 DMA is issued during rt==0 of the current head.
"""

import numpy as np

N = 4096
D_IN = 512
D_H = 2048
USED = 32
NCORES = 8
HPC = USED // NCORES  # heads per core = 4
KT = D_IN // 128      # k tiles = 4
RT = N // 128         # row tiles = 32
HB = D_H // 2         # psum half width = 1024

_PROGS = {}


def _build(mlo, mhi):
    import concourse.tile as tile
    import concourse.mybir as mybir
    from concourse import bacc

    f32 = mybir.dt.float32
    bf16 = mybir.dt.bfloat16
    mw = mhi - mlo

    nc = bacc.Bacc("TRN2", target_bir_lowering=False, debug=False)

    xT_d = nc.dram_tensor("xT", [KT, 128, N], bf16, kind="ExternalInput").ap()
    x_d = nc.dram_tensor("x", [N, D_IN], f32, kind="ExternalInput").ap()
    w1_d = nc.dram_tensor("w1", [HPC, 128, KT * D_H], bf16, kind="ExternalInput").ap()
    sb1_d = nc.dram_tensor("sb1", [128, HPC * D_H], bf16, kind="ExternalInput").ap()
    w2m_d = nc.dram_tensor("w2m", [128, HPC * mw], f32, kind="ExternalInput").ap()
    b2_d = nc.dram_tensor("b2r", [128, HPC], f32, kind="ExternalInput").ap()
    out_d = nc.dram_tensor("out", [N, HPC * 513], f32, kind="ExternalOutput").ap()
    y_d = nc.dram_tensor("yst", [HPC, 128, RT], f32, kind="ExternalOutput").ap()

    with tile.TileContext(nc) as tc:
        with tc.tile_pool(name="xt", bufs=1) as xtp, \
             tc.tile_pool(name="cst", bufs=1) as cst, \
             tc.tile_pool(name="w1p", bufs=2) as w1p, \
             tc.tile_pool(name="ys", bufs=2) as ysp, \
             tc.tile_pool(name="xc", bufs=4) as xcp, \
             tc.tile_pool(name="ps", bufs=4, space="PSUM") as pp, \
             tc.tile_pool(name="scr", bufs=3) as scr, \
             tc.tile_pool(name="sm", bufs=3) as smp, \
             tc.tile_pool(name="acc", bufs=6) as accp, \
             tc.tile_pool(name="j3", bufs=6) as j3p:

            xts = []
            for k in range(KT):
                t = xtp.tile([128, N], bf16, tag=f"x{k}")
                nc.sync.dma_start(t[:], xT_d[k])
                xts.append(t)
            sb1 = cst.tile([128, HPC * D_H], bf16, tag="sb1")
            nc.sync.dma_start(sb1[:], sb1_d[:])
            w2m = cst.tile([128, HPC * mw], f32, tag="w2m")
            nc.sync.dma_start(w2m[:], w2m_d[:])
            b2r = cst.tile([128, HPC], f32, tag="b2r")
            nc.sync.dma_start(b2r[:], b2_d[:])

            mx = mybir.AluOpType.max
            mn = mybir.AluOpType.min
            mult = mybir.AluOpType.mult
            ident = mybir.ActivationFunctionType.Identity

            w1ts = [w1p.tile([128, KT * D_H], bf16, tag="w1", name="w1t0")]
            nc.sync.dma_start(w1ts[0][:], w1_d[0])

            for h in range(HPC):
                w1t = w1ts[h]
                ys = ysp.tile([128, RT], f32, tag="ys", name=f"ys{h}")
                for rt in range(RT):
                    rs = rt * 128
                    if rt // 8 == h:
                        # x-block copies staged through SBUF, issued from the
                        # GpSimd engine (SW DGE) so DGE ring backpressure
                        # never stalls a compute engine's FIFO; spread across
                        # heads. DRAM->DRAM direct was 14x slower (25 GB/s).
                        xst = xcp.tile([128, 512], f32, tag="xst")
                        nc.gpsimd.dma_start(xst[:], x_d[rs:rs + 128, :])
                        for hh in range(HPC):
                            nc.gpsimd.dma_start(
                                out_d[rs:rs + 128, hh * 513:hh * 513 + 512],
                                xst[:],
                            )
                    ps_a = pp.tile([128, HB], f32, tag="ps", name=f"psa{h}_{rt}")
                    ps_b = pp.tile([128, HB], f32, tag="ps", name=f"psb{h}_{rt}")
                    for k in range(KT):
                        for tt in range(4):
                            ps = ps_a if tt < 2 else ps_b
                            nc.tensor.matmul(
                                ps[:, (tt % 2) * 512:(tt % 2) * 512 + 512],
                                lhsT=xts[k][:, rs:rs + 128],
                                rhs=w1t[:, k * D_H + tt * 512:k * D_H + (tt + 1) * 512],
                                start=(k == 0),
                                stop=(k == KT - 1),
                            )
                    if rt == 0 and h + 1 < HPC:
                        w1ts.append(w1p.tile([128, KT * D_H], bf16, tag="w1",
                                             name=f"w1t{h + 1}"))
                        nc.sync.dma_start(w1ts[h + 1][:], w1_d[h + 1])
                    c0 = h * D_H
                    acc = accp.tile([128, 3], f32, tag="acc")
                    sc = scr.tile([128, D_H], f32, tag="sc")
                    nc.vector.scalar_tensor_tensor(
                        out=sc[:, 0:HB],
                        in0=ps_a[:],
                        scalar=1.0,
                        in1=sb1[:, c0:c0 + HB],
                        op0=mult,
                        op1=mx,
                        accum_out=acc[:, 0:1],
                    )
                    nc.vector.scalar_tensor_tensor(
                        out=sc[:, HB:D_H],
                        in0=ps_b[:],
                        scalar=-1.0,
                        in1=sb1[:, c0 + HB:c0 + D_H],
                        op0=mult,
                        op1=mn,
                        accum_out=acc[:, 1:2],
                    )
                    sm = smp.tile([128, mw], f32, tag="sm")
                    nc.vector.scalar_tensor_tensor(
                        out=sm[:],
                        in0=sc[:, mlo:mhi],
                        scalar=1.0,
                        in1=w2m[:, h * mw:(h + 1) * mw],
                        op0=mult,
                        op1=mult,
                        accum_out=acc[:, 2:3],
                    )
                    j3 = j3p.tile([128, 3], f32, tag="j3")
                    nc.scalar.activation(
                        j3[:], acc[:], ident,
                        bias=b2r[:, h:h + 1], scale=1.0,
                        accum_out=ys[:, rt:rt + 1],
                    )
                nc.sync.dma_start(y_d[h], ys[:])

    nc.compile()
    return nc


def _get_program(mlo, mhi):
    key = (mlo, mhi)
    if key not in _PROGS:
        _PROGS[key] = _build(mlo, mhi)
    return _PROGS[key]


def kernel(x, W1, b1, W2, b2):
    import ml_dtypes
    from concourse.bass_utils import run_bass_kernel_spmd

    x = np.asarray(x, dtype=np.float32)
    W1 = np.asarray(W1, dtype=np.float32)
    b1 = np.asarray(b1, dtype=np.float32)
    W2 = np.asarray(W2, dtype=np.float32)
    b2 = np.asarray(b2, dtype=np.float32)

    # M region must contain every head's sign boundary; widen if needed
    P = (W2[:USED] > 0).sum(axis=1)
    mlo, mhi = 960, 1088
    if P.min() < mlo:
        mlo = max(0, int(P.min()) // 64 * 64)
    if P.max() > mhi:
        mhi = min(D_H, -(-int(P.max()) // 64) * 64)
    assert mlo < 1024 < mhi
    mw = mhi - mlo

    nc = _get_program(mlo, mhi)

    xT = np.ascontiguousarray(x.T).astype(ml_dtypes.bfloat16).reshape(KT, 128, N)

    in_maps = []
    for c in range(NCORES):
        w1heads = []
        sb1cols = np.empty(HPC * D_H, dtype=np.float32)
        w2mcols = np.empty(HPC * mw, dtype=np.float32)
        b2eff = np.empty(HPC, dtype=np.float32)
        for i in range(HPC):
            h = HPC * c + i
            w2 = W2[h]
            order = np.argsort(-w2, kind="stable")  # descending w2
            w2s = w2[order]
            b1s = b1[h][order]
            assert w2s[mlo - 1] > 0 and w2s[mhi] < 0, "region overflow"
            aw = np.abs(w2s)
            scale = aw.copy()
            scale[mlo:mhi] = 1.0  # M region stays unfolded
            w1s = W1[h][:, order] * scale[None, :]
            sb = np.empty(D_H, dtype=np.float32)
            sb[:mlo] = -aw[:mlo] * b1s[:mlo]
            sb[mlo:1024] = -b1s[mlo:1024]          # Ma (raw, in bigA)
            sb[1024:mhi] = b1s[1024:mhi]           # Mb (raw, in bigB)
            sb[mhi:] = aw[mhi:] * b1s[mhi:]
            sb1cols[i * D_H:(i + 1) * D_H] = sb
            w2mcols[i * mw:i * mw + (1024 - mlo)] = w2s[mlo:1024] - 1.0
            w2mcols[i * mw + (1024 - mlo):(i + 1) * mw] = -(w2s[1024:mhi] + 1.0)
            b2eff[i] = (
                b2[h].astype(np.float64)
                + np.dot(w2.astype(np.float64), b1[h].astype(np.float64))
            ) / 3.0
            # [512, 2048] -> [128 part, KT*D_H] with cols = k*D_H + j
            w1heads.append(
                np.ascontiguousarray(
                    w1s.reshape(KT, 128, D_H).transpose(1, 0, 2)
                ).reshape(128, KT * D_H)
            )
        in_maps.append({
            "xT": xT,
            "x": x,
            "w1": np.ascontiguousarray(
                np.stack(w1heads, axis=0).astype(ml_dtypes.bfloat16)
            ),
            "sb1": np.ascontiguousarray(
                np.broadcast_to(
                    sb1cols.reshape(1, -1).astype(ml_dtypes.bfloat16),
                    (128, HPC * D_H),
                )
            ),
            "w2m": np.ascontiguousarray(
                np.broadcast_to(w2mcols.reshape(1, -1), (128, HPC * mw))
            ),
            "b2r": np.ascontiguousarray(
                np.broadcast_to(b2eff.reshape(1, -1), (128, HPC))
            ),
        })

    import os
    trace = os.environ.get("BASS_KERNEL_TRACE") == "1"
    if trace:
        import sys
        sys.path.insert(0, "/tmp")
        try:
            import axon_shim
            axon_shim.install()
        except Exception:
            trace = False
    res = run_bass_kernel_spmd(nc, in_maps, list(range(NCORES)), trace=trace)
    kernel.last_result = res

    out = np.concatenate([res.results[c]["out"] for c in range(NCORES)], axis=1)
    for c in range(NCORES):
        yst = res.results[c]["yst"]  # [HPC, 128, RT]
        for i in range(HPC):
            # y[n] for n = rt*128 + p  lives at yst[i, p, rt]
            out[:, (c * HPC + i) * 513 + 512] = yst[i].T.reshape(N)
    return out


# revision 12
# speedup vs baseline: 2.1941x; 1.1174x over previous
"""Trainium2 Bass kernel for nn_DeepModel_multi_12945031430869.

Computes, for heads h in 0..31:
    y[:, h] = relu(x @ W1[h] + b1[h]) @ W2[h] + b2[h]
    out[:, h*513:(h+1)*513] = [x, y[:, h]]          # [4096, 16416]

Sharding: head-parallel across 8 NeuronCores (4 heads per core). Each core
produces its own [4096, 4*513] column block; the host concatenates them and
inserts the y columns.

Per-core device program (v4):
  - GEMM1 in bf16 on the PE array. Per (head, row-tile) two [128, 1024]
    PSUM half-tiles (2 banks each, pool bufs=4 -> all 8 banks). Matmul
    order is k-outer across BOTH halves so each stationary x-block feeds
    4 matmuls (fewer exposed weight reloads).
  - Epilogue folds |w2| into W1 columns (host), sorted by descending w2:
    cols [0,960) all-positive folded, [960,1088) mixed "M" region raw,
    [1088,2048) all-negative folded. Three DVE ops per (head, rt):
      bigA: (ps_a * +1) max sb1 -> sc[:, 0:1024],   accum -> accA
      bigB: (ps_b * -1) min sb1 -> sc[:, 1024:2048], accum -> accB
      M2:   sc[:, 960:1088] * v -> accum accM,  v = w2-1 (Ma) | -(w2+1) (Mb)
    Constant residues sum to b2eff = b2 + sum_f w2_f b1_f; applied by the
    Scalar engine: activation(Identity, bias=b2eff/3, accum_out) writes y
    into a per-head [128, 32] staging tile (col = rt) -> one DMA per head
    to the compact y_d DRAM tensor. Host scatters y into the out columns.
  - The output x-blocks never touch the compute pipeline: 128 independent
    DRAM->DRAM DMAs (issued spread across head 0's loop) copy x into the
    out blocks. This removes the ob-tile DMA-in -> activation -> DMA-out
    serialization that throttled v2/v3 (Sync-queue waits of 10-40 us).
  - Next head's W1 DMA is issued during rt==0 of the current head.
"""

import numpy as np

N = 4096
D_IN = 512
D_H = 2048
USED = 32
NCORES = 8
HPC = USED // NCORES  # heads per core = 4
KT = D_IN // 128      # k tiles = 4
RT = N // 128         # row tiles = 32
HB = D_H // 2         # psum half width = 1024

_PROGS = {}


def _build(mlo, mhi):
    import concourse.tile as tile
    import concourse.mybir as mybir
    from concourse import bacc

    f32 = mybir.dt.float32
    bf16 = mybir.dt.bfloat16
    mw = mhi - mlo

    nc = bacc.Bacc("TRN2", target_bir_lowering=False, debug=False)

    xT_d = nc.dram_tensor("xT", [KT, 128, N], bf16, kind="ExternalInput").ap()
    x_d = nc.dram_tensor("x", [N, D_IN], f32, kind="ExternalInput").ap()
    w1_d = nc.dram_tensor("w1", [HPC, 128, KT * D_H], bf16, kind="ExternalInput").ap()
    sb1_d = nc.dram_tensor("sb1", [128, HPC * D_H], bf16, kind="ExternalInput").ap()
    w2m_d = nc.dram_tensor("w2m", [128, HPC * mw], f32, kind="ExternalInput").ap()
    b2_d = nc.dram_tensor("b2r", [128, HPC], f32, kind="ExternalInput").ap()
    out_d = nc.dram_tensor("out", [N, HPC * 513], f32, kind="ExternalOutput").ap()
    y_d = nc.dram_tensor("yst", [HPC, 128, RT], f32, kind="ExternalOutput").ap()

    with tile.TileContext(nc) as tc:
        with tc.tile_pool(name="xt", bufs=1) as xtp, \
             tc.tile_pool(name="cst", bufs=1) as cst, \
             tc.tile_pool(name="w1p", bufs=2) as w1p, \
             tc.tile_pool(name="ys", bufs=2) as ysp, \
             tc.tile_pool(name="xc", bufs=4) as xcp, \
             tc.tile_pool(name="ps", bufs=4, space="PSUM") as pp, \
             tc.tile_pool(name="scr", bufs=3) as scr, \
             tc.tile_pool(name="sm", bufs=3) as smp, \
             tc.tile_pool(name="acc", bufs=6) as accp, \
             tc.tile_pool(name="j3", bufs=6) as j3p:

            # startup-latency-ordered preamble: only what the first row
            # tiles touch (xT cols 0:1024 = rt 0..7, w1 head 0, sb1 head 0)
            # is transferred before the first matmul can fire
            xts = []
            for k in range(KT):
                t = xtp.tile([128, N], bf16, tag=f"x{k}")
                nc.sync.dma_start(t[:, 0:1024], xT_d[k, :, 0:1024])
                xts.append(t)
            w1ts = [w1p.tile([128, KT * D_H], bf16, tag="w1", name="w1t0")]
            nc.sync.dma_start(w1ts[0][:], w1_d[0])
            sb1 = cst.tile([128, HPC * D_H], bf16, tag="sb1")
            nc.sync.dma_start(sb1[:, 0:D_H], sb1_d[:, 0:D_H])
            w2m = cst.tile([128, HPC * mw], f32, tag="w2m")
            nc.sync.dma_start(w2m[:], w2m_d[:])
            b2r = cst.tile([128, HPC], f32, tag="b2r")
            nc.sync.dma_start(b2r[:], b2_d[:])
            for k in range(KT):
                nc.sync.dma_start(xts[k][:, 1024:N], xT_d[k, :, 1024:N])
            nc.sync.dma_start(sb1[:, D_H:], sb1_d[:, D_H:])

            mx = mybir.AluOpType.max
            mn = mybir.AluOpType.min
            mult = mybir.AluOpType.mult
            ident = mybir.ActivationFunctionType.Identity

            for h in range(HPC):
                w1t = w1ts[h]
                ys = ysp.tile([128, RT], f32, tag="ys", name=f"ys{h}")
                for rt in range(RT):
                    rs = rt * 128
                    if h == min(rt // 11, 2):
                        # x-block copies staged through SBUF, issued from the
                        # GpSimd engine (SW DGE) so DGE ring backpressure
                        # never stalls a compute engine's FIFO; spread across
                        # heads. DRAM->DRAM direct was 14x slower (25 GB/s).
                        xst = xcp.tile([128, 512], f32, tag="xst")
                        nc.gpsimd.dma_start(xst[:], x_d[rs:rs + 128, :])
                        for hh in range(HPC):
                            nc.gpsimd.dma_start(
                                out_d[rs:rs + 128, hh * 513:hh * 513 + 512],
                                xst[:],
                            )
                    ps_a = pp.tile([128, HB], f32, tag="ps", name=f"psa{h}_{rt}")
                    ps_b = pp.tile([128, HB], f32, tag="ps", name=f"psb{h}_{rt}")
                    for k in range(KT):
                        for tt in range(4):
                            ps = ps_a if tt < 2 else ps_b
                            nc.tensor.matmul(
                                ps[:, (tt % 2) * 512:(tt % 2) * 512 + 512],
                                lhsT=xts[k][:, rs:rs + 128],
                                rhs=w1t[:, k * D_H + tt * 512:k * D_H + (tt + 1) * 512],
                                start=(k == 0),
                                stop=(k == KT - 1),
                            )
                    if rt == 0 and h + 1 < HPC:
                        w1ts.append(w1p.tile([128, KT * D_H], bf16, tag="w1",
                                             name=f"w1t{h + 1}"))
                        nc.sync.dma_start(w1ts[h + 1][:], w1_d[h + 1])
                    c0 = h * D_H
                    acc = accp.tile([128, 3], f32, tag="acc")
                    sc = scr.tile([128, D_H], f32, tag="sc")
                    nc.vector.scalar_tensor_tensor(
                        out=sc[:, 0:HB],
                        in0=ps_a[:],
                        scalar=1.0,
                        in1=sb1[:, c0:c0 + HB],
                        op0=mult,
                        op1=mx,
                        accum_out=acc[:, 0:1],
                    )
                    nc.vector.scalar_tensor_tensor(
                        out=sc[:, HB:D_H],
                        in0=ps_b[:],
                        scalar=-1.0,
                        in1=sb1[:, c0 + HB:c0 + D_H],
                        op0=mult,
                        op1=mn,
                        accum_out=acc[:, 1:2],
                    )
                    sm = smp.tile([128, mw], f32, tag="sm")
                    nc.vector.scalar_tensor_tensor(
                        out=sm[:],
                        in0=sc[:, mlo:mhi],
                        scalar=1.0,
                        in1=w2m[:, h * mw:(h + 1) * mw],
                        op0=mult,
                        op1=mult,
                        accum_out=acc[:, 2:3],
                    )
                    j3 = j3p.tile([128, 3], f32, tag="j3")
                    nc.scalar.activation(
                        j3[:], acc[:], ident,
                        bias=b2r[:, h:h + 1], scale=1.0,
                        accum_out=ys[:, rt:rt + 1],
                    )
                nc.sync.dma_start(y_d[h], ys[:])

    nc.compile()
    return nc


def _get_program(mlo, mhi):
    key = (mlo, mhi)
    if key not in _PROGS:
        _PROGS[key] = _build(mlo, mhi)
    return _PROGS[key]


def kernel(x, W1, b1, W2, b2):
    import ml_dtypes
    from concourse.bass_utils import run_bass_kernel_spmd

    x = np.asarray(x, dtype=np.float32)
    W1 = np.asarray(W1, dtype=np.float32)
    b1 = np.asarray(b1, dtype=np.float32)
    W2 = np.asarray(W2, dtype=np.float32)
    b2 = np.asarray(b2, dtype=np.float32)

    # M region must contain every head's sign boundary; widen if needed
    P = (W2[:USED] > 0).sum(axis=1)
    mlo, mhi = 960, 1088
    if P.min() < mlo:
        mlo = max(0, int(P.min()) // 64 * 64)
    if P.max() > mhi:
        mhi = min(D_H, -(-int(P.max()) // 64) * 64)
    assert mlo < 1024 < mhi
    mw = mhi - mlo

    nc = _get_program(mlo, mhi)

    xT = np.ascontiguousarray(x.T).astype(ml_dtypes.bfloat16).reshape(KT, 128, N)

    in_maps = []
    for c in range(NCORES):
        w1heads = []
        sb1cols = np.empty(HPC * D_H, dtype=np.float32)
        w2mcols = np.empty(HPC * mw, dtype=np.float32)
        b2eff = np.empty(HPC, dtype=np.float32)
        for i in range(HPC):
            h = HPC * c + i
            w2 = W2[h]
            order = np.argsort(-w2, kind="stable")  # descending w2
            w2s = w2[order]
            b1s = b1[h][order]
            assert w2s[mlo - 1] > 0 and w2s[mhi] < 0, "region overflow"
            aw = np.abs(w2s)
            scale = aw.copy()
            scale[mlo:mhi] = 1.0  # M region stays unfolded
            w1s = W1[h][:, order] * scale[None, :]
            sb = np.empty(D_H, dtype=np.float32)
            sb[:mlo] = -aw[:mlo] * b1s[:mlo]
            sb[mlo:1024] = -b1s[mlo:1024]          # Ma (raw, in bigA)
            sb[1024:mhi] = b1s[1024:mhi]           # Mb (raw, in bigB)
            sb[mhi:] = aw[mhi:] * b1s[mhi:]
            sb1cols[i * D_H:(i + 1) * D_H] = sb
            w2mcols[i * mw:i * mw + (1024 - mlo)] = w2s[mlo:1024] - 1.0
            w2mcols[i * mw + (1024 - mlo):(i + 1) * mw] = -(w2s[1024:mhi] + 1.0)
            b2eff[i] = (
                b2[h].astype(np.float64)
                + np.dot(w2.astype(np.float64), b1[h].astype(np.float64))
            ) / 3.0
            # [512, 2048] -> [128 part, KT*D_H] with cols = k*D_H + j
            w1heads.append(
                np.ascontiguousarray(
                    w1s.reshape(KT, 128, D_H).transpose(1, 0, 2)
                ).reshape(128, KT * D_H)
            )
        in_maps.append({
            "xT": xT,
            "x": x,
            "w1": np.ascontiguousarray(
                np.stack(w1heads, axis=0).astype(ml_dtypes.bfloat16)
            ),
            "sb1": np.ascontiguousarray(
                np.broadcast_to(
                    sb1cols.reshape(1, -1).astype(ml_dtypes.bfloat16),
                    (128, HPC * D_H),
                )
            ),
            "w2m": np.ascontiguousarray(
                np.broadcast_to(w2mcols.reshape(1, -1), (128, HPC * mw))
            ),
            "b2r": np.ascontiguousarray(
                np.broadcast_to(b2eff.reshape(1, -1), (128, HPC))
            ),
        })

    import os
    trace = os.environ.get("BASS_KERNEL_TRACE") == "1"
    if trace:
        import sys
        sys.path.insert(0, "/tmp")
        try:
            import axon_shim
            axon_shim.install()
        except Exception:
            trace = False
    res = run_bass_kernel_spmd(nc, in_maps, list(range(NCORES)), trace=trace)
    kernel.last_result = res

    out = np.concatenate([res.results[c]["out"] for c in range(NCORES)], axis=1)
    for c in range(NCORES):
        yst = res.results[c]["yst"]  # [HPC, 128, RT]
        for i in range(HPC):
            # y[n] for n = rt*128 + p  lives at yst[i, p, rt]
            out[:, (c * HPC + i) * 513 + 512] = yst[i].T.reshape(N)
    return out
